# revision 9
# baseline (speedup 1.0000x reference)
"""Triangular matmul C = triu(triu(A) @ triu(B)) on 8 TRN2 NeuronCores.

N=4096 fp32, viewed as a 32x32 grid of 128x128 blocks; the MAC work is the
block-tetrahedron {I <= K <= J} (5984 blocks of 128^3).

Sharding is 2D over the output: column "phases" (512-wide J-groups) are
split into two classes CLS = {0,3,5,6} / {1,2,4,7} carrying exactly half
the MACs each; within a class, rows go to 4 cores per ROWS_TBL.  Core
c = (row set c%4, class c//4).

Numerics: operands are rounded to bf16 on the host and each block product
is a single bf16 matmul accumulating in fp32 PSUM (rel err ~2e-3 vs the
fp32 reference; the harness gate is 2e-2).  C is staged to fp16 in SBUF
and upcast on the host.

v10 (uniform pre-Switch loads): every chunk has its OWN SBUF tile (all
operands are SBUF-resident), so loads carry no false deps and stream
back-to-back while the PE runs.  Measured on v8/v9: any instruction
placed inside a Switch arm starts ~5-7us late (arm-entry instruction
fetch stall), which starved the PE at startup and re-throttled the HAM
clock (k=4/8).  So ALL load DMAs are now emitted BEFORE the Switch as
an instruction stream identical for every core: B chunks on the SP
HWDGE ring, A chunks on the ACT ring (two rings arm in parallel).
Uniformity requires core-independent chunk edges: the host pads each
core's B pack so strip boundaries land on the shared edges, and every
core loads the full NA_MAX/WB_UNI pack (zero tail for smaller cores --
the garbage loads trail the real prefix and overlap compute).

Switch arms keep only: per-core matmuls (Tensor; its ~5us entry stall
is hidden behind the warmup burst), PSUM->fp16 evictions (DVE), and ALL
C stores on GpSimd/SWDGE -- whose arm-entry and exit branches measure
~70ns (Q7 fetch path), unlike Activation whose exit branch cost 4.7-6us
on the tail in v8/v9.

The kernel takes FULL (unsharded) inputs and returns the FULL output.
"""

import numpy as np

N = 4096
BLK = 128
NB = 32
PW = 512  # phase width in cols (4 blocks) = one fp32 PSUM bank
N_CORES = 8
MODE = "bf16-uniform-v10"

CLS = [[0, 3, 5, 6], [1, 2, 4, 7]]
# Row sets per class (4 cores each), from the assignment optimizer.
ROWS_TBL = [
    [[3, 7, 9, 14, 17, 20, 22, 24], [0, 6, 8, 10, 25, 26, 28, 31],
     [4, 11, 12, 13, 15, 16, 18, 23], [1, 2, 5, 19, 21, 27, 29, 30]],
    [[0, 1, 12, 24, 25, 26, 29, 31], [2, 3, 8, 13, 21, 22, 27, 30],
     [5, 7, 11, 14, 15, 16, 18, 20], [4, 6, 9, 10, 17, 19, 23, 28]],
]

N_WARM = 14  # dummy warmup matmuls (beat the HAM clock gate + entry stall)


def _b_edges():
    """Uniform B chunk edges: small first (arrival latency), then big
    (fewer issue instructions)."""
    e = [1024, 3072, 6144, 10240, 14336, 19456, 24576, 29696]
    while e[-1] < 60000:
        e.append(e[-1] + 5120)
    return e


def _a_cuts():
    cuts = [12, 44]
    while cuts[-1] < NA_MAX:
        cuts.append(min(cuts[-1] + 32, NA_MAX))
    return cuts


def _core_rs(c):
    return c % 4, c // 4


def _rows_of(c):
    r, s = _core_rs(c)
    return ROWS_TBL[s][r]


def _phases(c):
    """[(p, active_rows_desc)] in processing order: phases descending."""
    _, s = _core_rs(c)
    out = []
    for p in sorted(CLS[s], reverse=True):
        act = sorted((i for i in _rows_of(c) if i <= 4 * p + 3), reverse=True)
        if act:
            out.append((p, act))
    return out


def _strips_desc(p, m):
    """K-strips (q, col0, width_cols) of phase p, q descending to m."""
    out = []
    for q in range(4 * p + 3, m - 1, -1):
        c0 = max(4 * p, q) * BLK
        out.append((q, c0, (4 * p + 4) * BLK - c0))
    return out


def _b_layout(c):
    """bpack in global consumption order: phases big-first, strips q-desc,
    padded so no strip crosses a uniform chunk edge.
    Returns ({(p, q): col offset}, padded total width)."""
    edges = _b_edges()
    off, w, ei = {}, 0, 0
    for p, act in _phases(c):
        for q, _, wid in _strips_desc(p, act[-1]):
            while ei < len(edges) and w + wid > edges[ei]:
                if w < edges[ei]:
                    w = edges[ei]  # pad to the edge
                ei += 1
            off[(p, q)] = w
            w += wid
    return off, w


def _a_layout(c):
    """Packed-A slots in consumption order: rows descending (first phase
    order), K ascending within a row."""
    phs = _phases(c)
    kmax = 4 * phs[0][0] + 3
    rows = sorted(set(i for _, act in phs for i in act), reverse=True)
    slots = {}
    for i in rows:
        for q in range(i, kmax + 1):
            slots[(q, i)] = len(slots)
    return slots


def _c_layout(c):
    """Packed-C 512-col slots: {(p, i): slot}, contiguous per phase."""
    slots = {}
    for p, act in _phases(c):
        for i in act:
            slots[(p, i)] = len(slots)
    return slots


NA_MAX = max(len(_a_layout(c)) for c in range(N_CORES))
WB_UNI = max(_b_layout(c)[1] for c in range(N_CORES))
NC_MAX = max(len(_c_layout(c)) for c in range(N_CORES))
B_CUTS = [0] + [e for e in _b_edges() if e < WB_UNI] + [WB_UNI]
A_CUTS = [0] + _a_cuts()


class _Chunks:
    """Resolve a packed offset to (tile, local offset)."""

    def __init__(self, cuts, tiles):
        self.cuts = cuts
        self.tiles = tiles  # tiles[k] covers [cuts[k], cuts[k+1])

    def at(self, off):
        for k in range(len(self.tiles)):
            if self.cuts[k] <= off < self.cuts[k + 1]:
                return self.tiles[k], off - self.cuts[k]
        raise AssertionError(f"offset {off} outside chunks {self.cuts}")


def _emit_preamble(nc, tc, pools, dram_io):
    """Identical for all cores, before the Switch: PE warmup + the WHOLE
    load stream.  B chunks on the SP ring, A chunks on the ACT ring: the
    rings arm in parallel and no load issue ever sits behind a Switch
    arm-entry fetch stall."""
    import concourse.mybir as mybir

    f32 = mybir.dt.float32
    bf16 = mybir.dt.bfloat16
    apool, bpool, cpool, psum_pool = pools
    apack, bpack = dram_io["apack"], dram_io["bpack"]

    warm = bpool.tile([BLK, PW], bf16, name="warm", tag="warm")
    nc.gpsimd.memset(warm[:], 0.0)
    wps = psum_pool.tile([BLK, PW], f32, name="warmps", tag="ps7")
    for i in range(N_WARM):
        nc.tensor.matmul(
            wps[:], warm[:, :BLK], warm[:], start=(i == 0), stop=(i == N_WARM - 1)
        )
    b_tiles, a_tiles = [], []
    nb, na = len(B_CUTS) - 1, len(A_CUTS) - 1
    for k in range(max(nb, na)):
        if k < nb:
            lo, hi = B_CUTS[k], B_CUTS[k + 1]
            t = bpool.tile([BLK, hi - lo], bf16, name=f"b{k}", tag=f"b{k}")
            nc.sync.dma_start(t[:], bpack[:, lo:hi])
            b_tiles.append(t)
        if k < na:
            lo, hi = A_CUTS[k], A_CUTS[k + 1]
            t = apool.tile([BLK, hi - lo, BLK], bf16, name=f"a{k}", tag=f"a{k}")
            nc.scalar.dma_start(t[:], apack[:, lo:hi, :])
            a_tiles.append(t)
    return _Chunks(A_CUTS, a_tiles), _Chunks(B_CUTS, b_tiles)


def _emit_core(nc, tc, pools, dram_io, core, ach, bch):
    import concourse.mybir as mybir

    f32 = mybir.dt.float32
    fp16 = mybir.dt.float16
    apool, bpool, cpool, psum_pool = pools
    cpack = dram_io["cpack"]
    aslot = _a_layout(core)
    cslot = _c_layout(core)
    boff, _ = _b_layout(core)
    phs = _phases(core)

    seq = [(pi, p, act, i) for pi, (p, act) in enumerate(phs) for i in act]

    # compute: phases big-first, rows descending, per-row eviction.  ALL
    # C stores ride SWDGE (GpSimd): its arm branches cost ~70ns, unlike
    # the Activation ring whose exit branch cost 4.7-6us on the tail.
    bank = 0
    cst = {}
    for j, (pi, p, act, i) in enumerate(seq):
        last_strip = 4 * p + 3
        last_phase = pi == len(phs) - 1
        if not last_phase and p not in cst:
            cst[p] = cpool.tile(
                [BLK, len(act) * PW], fp16, name=f"cst_{p}", tag=f"cst{pi}"
            )
        pst = psum_pool.tile([BLK, PW], f32, name=f"ps_{p}_{i}", tag=f"ps{bank % 8}")
        bank += 1
        for q, c0, wid in reversed(_strips_desc(p, i)):
            rel = c0 - 4 * p * BLK
            bt, blo = bch.at(boff[(p, q)])
            at_, alo = ach.at(aslot[(q, i)])
            nc.tensor.matmul(
                pst[:, rel : rel + wid],
                at_[:, alo, :],
                bt[:, blo : blo + wid],
                start=(q == i),
                stop=(q == last_strip),
            )
        mr = max(0, i - 4 * p) * BLK
        if last_phase:
            ji = act.index(i)
            ct = cpool.tile([BLK, PW], fp16, name=f"ct_{i}", tag=f"ct{ji % 4}")
            nc.vector.tensor_copy(ct[:, mr:PW], pst[:, mr:PW])
            nc.gpsimd.dma_start(
                cpack[:, cslot[(p, i)] * PW + mr : (cslot[(p, i)] + 1) * PW],
                ct[:, mr:PW],
            )
        else:
            s0 = cslot[(p, act[0])]
            jrow = cslot[(p, i)] - s0
            nc.vector.tensor_copy(
                cst[p][:, jrow * PW + mr : (jrow + 1) * PW], pst[:, mr:PW]
            )
            if i == act[-1]:
                nc.gpsimd.dma_start(
                    cpack[:, s0 * PW : (s0 + len(act)) * PW], cst[p][:]
                )


def _build():
    import concourse.mybir as mybir
    import concourse.tile as tile
    from concourse import bacc

    nc = bacc.Bacc(None, target_bir_lowering=False, debug=False)
    bf16 = mybir.dt.bfloat16
    fp16 = mybir.dt.float16
    with tile.TileContext(nc) as tc:
        with (
            tc.tile_pool(name="dram", bufs=1, space="DRAM") as dram,
            tc.tile_pool(name="apool", bufs=1) as apool,
            tc.tile_pool(name="bpool", bufs=1) as bpool,
            tc.tile_pool(name="cpool", bufs=1) as cpool,
            tc.tile_pool(name="psum", bufs=1, space="PSUM") as psum_pool,
        ):
            dram_io = {
                "apack": dram.tile(
                    [BLK, NA_MAX, BLK], bf16, kind="ExternalInput",
                    name="apack", uniquify=False,
                ),
                "bpack": dram.tile(
                    [BLK, WB_UNI], bf16, kind="ExternalInput",
                    name="bpack", uniquify=False,
                ),
                "cpack": dram.tile(
                    [BLK, NC_MAX * PW], fp16, kind="ExternalOutput",
                    name="cpack", uniquify=False,
                ),
            }
            pid = nc.partition_id()
            pools = (apool, bpool, cpool, psum_pool)
            arm_engines = [
                e for e in mybir.ALL_ENGINES if e.name in ("PE", "DVE", "Pool")
            ]
            tc.switch_hint({e: pid for e in arm_engines}, N_CORES, label="core")
            ach, bch = _emit_preamble(nc, tc, pools, dram_io)
            for c in tc.Switch(pid, N_CORES, hint="core"):
                _emit_core(nc, tc, pools, dram_io, c, ach, bch)
    nc.compile()
    return nc


_cached_nc = None

# Optional profiling knobs (used by test.py; harness leaves them off).
TRACE = False
TRACE_KW = {}
LAST_RESULTS = None


def _get_nc():
    global _cached_nc
    if _cached_nc is None:
        _cached_nc = _build()
    return _cached_nc


def _host_pack(A, B):
    import ml_dtypes

    bf16 = ml_dtypes.bfloat16
    AT = np.ascontiguousarray(A.T).astype(bf16)
    Bb = B.astype(bf16)
    apacks, bpacks = [], []
    for c in range(N_CORES):
        ap = np.zeros((BLK, NA_MAX, BLK), dtype=bf16)
        for (q, i), idx in _a_layout(c).items():
            ap[:, idx, :] = AT[q * BLK : (q + 1) * BLK, i * BLK : (i + 1) * BLK]
        bp = np.zeros((BLK, WB_UNI), dtype=bf16)
        boff, _ = _b_layout(c)
        for p, act in _phases(c):
            for q, c0, wid in _strips_desc(p, act[-1]):
                w0 = boff[(p, q)]
                bp[:, w0 : w0 + wid] = Bb[q * BLK : (q + 1) * BLK, c0 : c0 + wid]
        apacks.append(ap)
        bpacks.append(bp)
    return apacks, bpacks


def kernel(A, B):
    from concourse.bass_utils import run_bass_kernel_spmd

    A = np.asarray(A, dtype=np.float32)
    B = np.asarray(B, dtype=np.float32)
    nc = _get_nc()
    apacks, bpacks = _host_pack(A, B)
    in_maps = [{"apack": apacks[c], "bpack": bpacks[c]} for c in range(N_CORES)]
    res = run_bass_kernel_spmd(
        nc, in_maps, core_ids=list(range(N_CORES)), trace=TRACE, **TRACE_KW
    )
    global LAST_RESULTS
    LAST_RESULTS = res

    C = np.zeros((N, N), dtype=np.float32)
    for c in range(N_CORES):
        cp = res.results[c]["cpack"]
        for (p, i), j in _c_layout(c).items():
            mr = max(0, i - 4 * p) * BLK
            C[i * BLK : (i + 1) * BLK, p * PW + mr : (p + 1) * PW] = cp[
                :, j * PW + mr : (j + 1) * PW
            ].astype(np.float32)
    return np.triu(C)


# revision 14
# speedup vs baseline: 1.1197x; 1.1197x over previous
"""Triangular matmul C = triu(triu(A) @ triu(B)) on 8 TRN2 NeuronCores.

N=4096 fp32, viewed as a 32x32 grid of 128x128 blocks; the MAC work is the
block-tetrahedron {I <= K <= J} (5984 blocks of 128^3).

Sharding is 2D over the output: column "phases" (512-wide J-groups) are
split into two classes CLS = {0,3,5,6} / {1,2,4,7} carrying exactly half
the MACs each; within a class, rows go to 4 cores per ROWS_TBL.  Core
c = (row set c%4, class c//4).

Numerics: operands are rounded to bf16 on the host and each block product
is a single bf16 matmul accumulating in fp32 PSUM (rel err ~2e-3 vs the
fp32 reference; the harness gate is 2e-2).  C is staged to fp16 in SBUF
and upcast on the host.

v10 (uniform pre-Switch loads): every chunk has its OWN SBUF tile (all
operands are SBUF-resident), so loads carry no false deps and stream
back-to-back while the PE runs.  Measured on v8/v9: any instruction
placed inside a Switch arm starts ~5-7us late (arm-entry instruction
fetch stall), which starved the PE at startup and re-throttled the HAM
clock (k=4/8).  So ALL load DMAs are now emitted BEFORE the Switch as
an instruction stream identical for every core: B chunks on the SP
HWDGE ring, A chunks on the ACT ring (two rings arm in parallel).
Uniformity requires core-independent chunk edges: the host pads each
core's B pack so strip boundaries land on the shared edges, and every
core loads the full NA_MAX/WB_UNI pack (zero tail for smaller cores --
the garbage loads trail the real prefix and overlap compute).

Switch arms keep only: per-core matmuls (Tensor; its ~5us entry stall
is hidden behind the warmup burst), PSUM->fp16 evictions (DVE), and ALL
C stores on GpSimd/SWDGE -- whose arm-entry and exit branches measure
~70ns (Q7 fetch path), unlike Activation whose exit branch cost 4.7-6us
on the tail in v8/v9.

v11 (K-wavefront schedule): v10's DMA stream was clean (~420 GB/s,
loads done by t=40us) but the PE still starved mid-startup: row-major
order within a phase means the 3rd-4th row already sweeps ~75% of the
phase's B columns, far ahead of the stream.  The per-phase loop is now
inverted to strip-wavefront order -- strips processed exactly in pack
(= arrival) order, each strip applied to ALL active rows (one matmul
per row into that row's dedicated PSUM bank; <=8 rows per phase = 8
banks, bank = row's index in the core's active-row list so cross-phase
bank reuse only waits on the SAME row's earlier eviction).  PE work per
loaded strip is then 4-8x its single-row cost, so the PE can never
outrun the stream, in any phase.  A row's accumulation region grows
leftward over the first three (diagonal) strips, so each such strip
emits an extra 128-col start=True matmul for the newly-opened sliver
(same total columns; the extra LDWEIGHTS hides under the 4-XBUS FWL).
Rows retire at strip q==i (mid-phase), spreading DVE evictions and C
stores instead of bunching them at phase end.

The kernel takes FULL (unsharded) inputs and returns the FULL output.
"""

import numpy as np

N = 4096
BLK = 128
NB = 32
PW = 512  # phase width in cols (4 blocks) = one fp32 PSUM bank
N_CORES = 8
MODE = "bf16-uniform-v10"

CLS = [[0, 3, 5, 6], [1, 2, 4, 7]]
# Row sets per class (4 cores each), from the assignment optimizer.
ROWS_TBL = [
    [[3, 7, 9, 14, 17, 20, 22, 24], [0, 6, 8, 10, 25, 26, 28, 31],
     [4, 11, 12, 13, 15, 16, 18, 23], [1, 2, 5, 19, 21, 27, 29, 30]],
    [[0, 1, 12, 24, 25, 26, 29, 31], [2, 3, 8, 13, 21, 22, 27, 30],
     [5, 7, 11, 14, 15, 16, 18, 20], [4, 6, 9, 10, 17, 19, 23, 28]],
]

N_WARM = 9  # dummy warmup matmuls: ~3.8us at the cold 1.2 GHz clock burns
# the HAM SHORT window and covers the Tensor arm-entry fetch stall, and
# ends before the first in-arm matmul needs the warmup PSUM bank (WAW).


def _b_edges():
    """Uniform B chunk edges: small first (arrival latency), then big
    (fewer issue instructions)."""
    e = [1024, 3072, 6144, 10240, 14336, 19456, 24576, 29696]
    while e[-1] < 60000:
        e.append(e[-1] + 5120)
    return e


def _a_cuts():
    cuts = [12, 44]
    while cuts[-1] < NA_MAX:
        cuts.append(min(cuts[-1] + 32, NA_MAX))
    return cuts


def _core_rs(c):
    return c % 4, c // 4


def _rows_of(c):
    r, s = _core_rs(c)
    return ROWS_TBL[s][r]


def _phases(c):
    """[(p, active_rows_desc)] in processing order: phases descending."""
    _, s = _core_rs(c)
    out = []
    for p in sorted(CLS[s], reverse=True):
        act = sorted((i for i in _rows_of(c) if i <= 4 * p + 3), reverse=True)
        if act:
            out.append((p, act))
    return out


def _strips_desc(p, m):
    """K-strips (q, col0, width_cols) of phase p, q descending to m."""
    out = []
    for q in range(4 * p + 3, m - 1, -1):
        c0 = max(4 * p, q) * BLK
        out.append((q, c0, (4 * p + 4) * BLK - c0))
    return out


def _b_layout(c):
    """bpack in global consumption order: phases big-first, strips q-desc,
    padded so no strip crosses a uniform chunk edge.
    Returns ({(p, q): col offset}, padded total width)."""
    edges = _b_edges()
    off, w, ei = {}, 0, 0
    for p, act in _phases(c):
        for q, _, wid in _strips_desc(p, act[-1]):
            while ei < len(edges) and w + wid > edges[ei]:
                if w < edges[ei]:
                    w = edges[ei]  # pad to the edge
                ei += 1
            off[(p, q)] = w
            w += wid
    return off, w


def _a_layout(c):
    """Packed-A slots in consumption order: everything is first used in
    the first (biggest) phase, in wavefront order: q descending, rows
    descending within a wavefront."""
    phs = _phases(c)
    kmax = 4 * phs[0][0] + 3
    rows = sorted(set(i for _, act in phs for i in act), reverse=True)
    slots = {}
    for q in range(kmax, -1, -1):
        for i in rows:
            if i <= q:
                slots[(q, i)] = len(slots)
    return slots


def _c_layout(c):
    """Packed-C 512-col slots: {(p, i): slot}, contiguous per phase."""
    slots = {}
    for p, act in _phases(c):
        for i in act:
            slots[(p, i)] = len(slots)
    return slots


NA_MAX = max(len(_a_layout(c)) for c in range(N_CORES))
WB_UNI = max(_b_layout(c)[1] for c in range(N_CORES))
NC_MAX = max(len(_c_layout(c)) for c in range(N_CORES))
B_CUTS = [0] + [e for e in _b_edges() if e < WB_UNI] + [WB_UNI]
A_CUTS = [0] + _a_cuts()


class _Chunks:
    """Resolve a packed offset to (tile, local offset)."""

    def __init__(self, cuts, tiles):
        self.cuts = cuts
        self.tiles = tiles  # tiles[k] covers [cuts[k], cuts[k+1])

    def at(self, off):
        for k in range(len(self.tiles)):
            if self.cuts[k] <= off < self.cuts[k + 1]:
                return self.tiles[k], off - self.cuts[k]
        raise AssertionError(f"offset {off} outside chunks {self.cuts}")


def _emit_preamble(nc, tc, pools, dram_io):
    """Identical for all cores, before the Switch: PE warmup + the WHOLE
    load stream.  B chunks on the SP ring, A chunks on the ACT ring: the
    rings arm in parallel and no load issue ever sits behind a Switch
    arm-entry fetch stall."""
    import concourse.mybir as mybir

    f32 = mybir.dt.float32
    bf16 = mybir.dt.bfloat16
    apool, bpool, cpool, psum_pool = pools
    apack, bpack = dram_io["apack"], dram_io["bpack"]

    warm = bpool.tile([BLK, PW], bf16, name="warm", tag="warm")
    nc.gpsimd.memset(warm[:], 0.0)
    wps = psum_pool.tile([BLK, PW], f32, name="warmps", tag="ps7")
    for i in range(N_WARM):
        nc.tensor.matmul(
            wps[:], warm[:, :BLK], warm[:], start=(i == 0), stop=(i == N_WARM - 1)
        )
    b_tiles, a_tiles = [], []
    nb, na = len(B_CUTS) - 1, len(A_CUTS) - 1
    for k in range(max(nb, na)):
        if k < nb:
            lo, hi = B_CUTS[k], B_CUTS[k + 1]
            t = bpool.tile([BLK, hi - lo], bf16, name=f"b{k}", tag=f"b{k}")
            nc.sync.dma_start(t[:], bpack[:, lo:hi])
            b_tiles.append(t)
        if k < na:
            lo, hi = A_CUTS[k], A_CUTS[k + 1]
            t = apool.tile([BLK, hi - lo, BLK], bf16, name=f"a{k}", tag=f"a{k}")
            nc.scalar.dma_start(t[:], apack[:, lo:hi, :])
            a_tiles.append(t)
    return _Chunks(A_CUTS, a_tiles), _Chunks(B_CUTS, b_tiles)


def _emit_core(nc, tc, pools, dram_io, core, ach, bch):
    import concourse.mybir as mybir

    f32 = mybir.dt.float32
    fp16 = mybir.dt.float16
    apool, bpool, cpool, psum_pool = pools
    cpack = dram_io["cpack"]
    aslot = _a_layout(core)
    cslot = _c_layout(core)
    boff, _ = _b_layout(core)
    phs = _phases(core)

    allrows = sorted(set(i for _, act in phs for i in act), reverse=True)
    bank_of = {i: k for k, i in enumerate(allrows)}  # stable across phases

    # compute: phases big-first; within a phase, strips in wavefront
    # (pack/arrival) order, each applied to all active rows.  A row's
    # region grows leftward over the diagonal strips, so each of those
    # emits an extra start=True matmul for the new 128-col sliver.  ALL
    # C stores ride SWDGE (GpSimd): its arm branches cost ~70ns, unlike
    # the Activation ring whose exit branch cost 4.7-6us on the tail.
    for pi, (p, act) in enumerate(phs):
        last_phase = pi == len(phs) - 1
        if not last_phase:
            cst = cpool.tile(
                [BLK, len(act) * PW], fp16, name=f"cst_{p}", tag=f"cst{pi}"
            )
        pst = {
            i: psum_pool.tile(
                [BLK, PW], f32, name=f"ps_{p}_{i}", tag=f"ps{bank_of[i]}"
            )
            for i in act
        }
        first = 4 * p + 3
        for q, c0, wid in _strips_desc(p, act[-1]):
            rel = c0 - 4 * p * BLK
            bt, blo = bch.at(boff[(p, q)])
            rows_in = [i for i in act if i <= q]
            rows_in = rows_in[1:] + rows_in[:1]  # retiring row (i == q) last
            for i in rows_in:
                at_, alo = ach.at(aslot[(q, i)])
                # start=True clears the whole bank's has_written bits, so
                # it is only legal on the group-opening strip; afterwards
                # the per-element bits overwrite-vs-accumulate correctly
                # as the region grows leftward over the diagonal strips.
                nc.tensor.matmul(
                    pst[i][:, rel : rel + wid],
                    at_[:, alo, :],
                    bt[:, blo : blo + wid],
                    start=(q == first),
                    stop=(q == i),
                )
            if q in act:  # row q retires: evict + store
                i = q
                mr = max(0, i - 4 * p) * BLK
                if last_phase:
                    ji = act.index(i)
                    ct = cpool.tile(
                        [BLK, PW], fp16, name=f"ct_{i}", tag=f"ct{ji % 4}"
                    )
                    nc.vector.tensor_copy(ct[:, mr:PW], pst[i][:, mr:PW])
                    nc.gpsimd.dma_start(
                        cpack[
                            :, cslot[(p, i)] * PW + mr : (cslot[(p, i)] + 1) * PW
                        ],
                        ct[:, mr:PW],
                    )
                else:
                    s0 = cslot[(p, act[0])]
                    jrow = cslot[(p, i)] - s0
                    nc.vector.tensor_copy(
                        cst[:, jrow * PW + mr : (jrow + 1) * PW], pst[i][:, mr:PW]
                    )
                    if i == act[-1]:
                        nc.gpsimd.dma_start(
                            cpack[:, s0 * PW : (s0 + len(act)) * PW], cst[:]
                        )


def _build():
    import concourse.mybir as mybir
    import concourse.tile as tile
    from concourse import bacc

    nc = bacc.Bacc(None, target_bir_lowering=False, debug=False)
    bf16 = mybir.dt.bfloat16
    fp16 = mybir.dt.float16
    with tile.TileContext(nc) as tc:
        with (
            tc.tile_pool(name="dram", bufs=1, space="DRAM") as dram,
            tc.tile_pool(name="apool", bufs=1) as apool,
            tc.tile_pool(name="bpool", bufs=1) as bpool,
            tc.tile_pool(name="cpool", bufs=1) as cpool,
            tc.tile_pool(name="psum", bufs=1, space="PSUM") as psum_pool,
        ):
            dram_io = {
                "apack": dram.tile(
                    [BLK, NA_MAX, BLK], bf16, kind="ExternalInput",
                    name="apack", uniquify=False,
                ),
                "bpack": dram.tile(
                    [BLK, WB_UNI], bf16, kind="ExternalInput",
                    name="bpack", uniquify=False,
                ),
                "cpack": dram.tile(
                    [BLK, NC_MAX * PW], fp16, kind="ExternalOutput",
                    name="cpack", uniquify=False,
                ),
            }
            pid = nc.partition_id()
            pools = (apool, bpool, cpool, psum_pool)
            arm_engines = [
                e for e in mybir.ALL_ENGINES if e.name in ("PE", "DVE", "Pool")
            ]
            tc.switch_hint({e: pid for e in arm_engines}, N_CORES, label="core")
            ach, bch = _emit_preamble(nc, tc, pools, dram_io)
            for c in tc.Switch(pid, N_CORES, hint="core"):
                _emit_core(nc, tc, pools, dram_io, c, ach, bch)
    nc.compile()
    return nc


_cached_nc = None

# Optional profiling knobs (used by test.py; harness leaves them off).
TRACE = False
TRACE_KW = {}
LAST_RESULTS = None


def _get_nc():
    global _cached_nc
    if _cached_nc is None:
        _cached_nc = _build()
    return _cached_nc


def _host_pack(A, B):
    import ml_dtypes

    bf16 = ml_dtypes.bfloat16
    AT = np.ascontiguousarray(A.T).astype(bf16)
    Bb = B.astype(bf16)
    apacks, bpacks = [], []
    for c in range(N_CORES):
        ap = np.zeros((BLK, NA_MAX, BLK), dtype=bf16)
        for (q, i), idx in _a_layout(c).items():
            ap[:, idx, :] = AT[q * BLK : (q + 1) * BLK, i * BLK : (i + 1) * BLK]
        bp = np.zeros((BLK, WB_UNI), dtype=bf16)
        boff, _ = _b_layout(c)
        for p, act in _phases(c):
            for q, c0, wid in _strips_desc(p, act[-1]):
                w0 = boff[(p, q)]
                bp[:, w0 : w0 + wid] = Bb[q * BLK : (q + 1) * BLK, c0 : c0 + wid]
        apacks.append(ap)
        bpacks.append(bp)
    return apacks, bpacks


def kernel(A, B):
    from concourse.bass_utils import run_bass_kernel_spmd

    A = np.asarray(A, dtype=np.float32)
    B = np.asarray(B, dtype=np.float32)
    nc = _get_nc()
    apacks, bpacks = _host_pack(A, B)
    in_maps = [{"apack": apacks[c], "bpack": bpacks[c]} for c in range(N_CORES)]
    res = run_bass_kernel_spmd(
        nc, in_maps, core_ids=list(range(N_CORES)), trace=TRACE, **TRACE_KW
    )
    global LAST_RESULTS
    LAST_RESULTS = res

    C = np.zeros((N, N), dtype=np.float32)
    for c in range(N_CORES):
        cp = res.results[c]["cpack"]
        for (p, i), j in _c_layout(c).items():
            mr = max(0, i - 4 * p) * BLK
            C[i * BLK : (i + 1) * BLK, p * PW + mr : (p + 1) * PW] = cp[
                :, j * PW + mr : (j + 1) * PW
            ].astype(np.float32)
    return np.triu(C)


# revision 16
# speedup vs baseline: 1.1658x; 1.0411x over previous
"""Triangular matmul C = triu(triu(A) @ triu(B)) on 8 TRN2 NeuronCores.

N=4096 fp32, viewed as a 32x32 grid of 128x128 blocks; the MAC work is the
block-tetrahedron {I <= K <= J} (5984 blocks of 128^3).

Sharding is 2D over the output: column "phases" (512-wide J-groups) are
split into two classes CLS = {0,3,5,6} / {1,2,4,7} carrying exactly half
the MACs each; within a class, rows go to 4 cores per ROWS_TBL.  Core
c = (row set c%4, class c//4).

Numerics: operands are rounded to bf16 on the host and each block product
is a single bf16 matmul accumulating in fp32 PSUM (rel err ~2e-3 vs the
fp32 reference; the harness gate is 2e-2).  C is staged to fp16 in SBUF
and upcast on the host.

v10 (uniform pre-Switch loads): every chunk has its OWN SBUF tile (all
operands are SBUF-resident), so loads carry no false deps and stream
back-to-back while the PE runs.  Measured on v8/v9: any instruction
placed inside a Switch arm starts ~5-7us late (arm-entry instruction
fetch stall), which starved the PE at startup and re-throttled the HAM
clock (k=4/8).  So ALL load DMAs are now emitted BEFORE the Switch as
an instruction stream identical for every core: B chunks on the SP
HWDGE ring, A chunks on the ACT ring (two rings arm in parallel).
Uniformity requires core-independent chunk edges: the host pads each
core's B pack so strip boundaries land on the shared edges, and every
core loads the full NA_MAX/WB_UNI pack (zero tail for smaller cores --
the garbage loads trail the real prefix and overlap compute).

Switch arms keep only: per-core matmuls (Tensor; its ~5us entry stall
is hidden behind the warmup burst), PSUM->fp16 evictions (DVE), and ALL
C stores on GpSimd/SWDGE -- whose arm-entry and exit branches measure
~70ns (Q7 fetch path), unlike Activation whose exit branch cost 4.7-6us
on the tail in v8/v9.

v11 (K-wavefront schedule): v10's DMA stream was clean (~420 GB/s,
loads done by t=40us) but the PE still starved mid-startup: row-major
order within a phase means the 3rd-4th row already sweeps ~75% of the
phase's B columns, far ahead of the stream.  The per-phase loop is now
inverted to strip-wavefront order -- strips processed exactly in pack
(= arrival) order, each strip applied to ALL active rows (one matmul
per row into that row's dedicated PSUM bank; <=8 rows per phase = 8
banks, bank = row's index in the core's active-row list so cross-phase
bank reuse only waits on the SAME row's earlier eviction).  PE work per
loaded strip is then 4-8x its single-row cost, so the PE can never
outrun the stream, in any phase.  A row's accumulation region grows
leftward over the first three (diagonal) strips, so each such strip
emits an extra 128-col start=True matmul for the newly-opened sliver
(same total columns; the extra LDWEIGHTS hides under the 4-XBUS FWL).
Rows retire at strip q==i (mid-phase), spreading DVE evictions and C
stores instead of bunching them at phase end.

The kernel takes FULL (unsharded) inputs and returns the FULL output.
"""

import numpy as np

N = 4096
BLK = 128
NB = 32
PW = 512  # phase width in cols (4 blocks) = one fp32 PSUM bank
N_CORES = 8
MODE = "bf16-uniform-v10"

CLS = [[0, 3, 5, 6], [1, 2, 4, 7]]
# Row sets per class (4 cores each), from the assignment optimizer:
# minimizes max-core blocks (758) and the uniform-load maxes (NA_MAX
# 135, WB ~33.8K cols).
ROWS_TBL = [
    [[1, 4, 5, 14, 22, 27, 29, 31], [0, 8, 10, 16, 17, 20, 21, 28],
     [3, 7, 9, 13, 18, 19, 25, 26], [2, 6, 11, 12, 15, 23, 24, 30]],
    [[7, 8, 9, 10, 11, 22, 23, 31], [1, 4, 13, 16, 17, 19, 21, 30],
     [0, 5, 6, 18, 20, 24, 27, 29], [2, 3, 12, 14, 15, 25, 26, 28]],
]

N_WARM = 16  # dummy warmup matmuls: burn the HAM SHORT window (first
# ~3.4us at 1.2 GHz) AND keep the PE busy through the Tensor arm-entry
# instruction-fetch stall (~5us, ends ~17us) so the MID window never
# sees a >3.4us idle and re-throttles.


def _b_edges():
    """Uniform B chunk edges: small first (arrival latency), then big
    (fewer issue instructions)."""
    e = [1024, 3072, 6144, 10240, 14336, 19456, 24576, 29696]
    while e[-1] < 60000:
        e.append(e[-1] + 5120)
    return e


def _a_cuts():
    cuts = [12, 44]
    while cuts[-1] < NA_MAX:
        cuts.append(min(cuts[-1] + 32, NA_MAX))
    return cuts


def _core_rs(c):
    return c % 4, c // 4


def _rows_of(c):
    r, s = _core_rs(c)
    return ROWS_TBL[s][r]


def _phases(c):
    """[(p, active_rows_desc)] in processing order: phases descending."""
    _, s = _core_rs(c)
    out = []
    for p in sorted(CLS[s], reverse=True):
        act = sorted((i for i in _rows_of(c) if i <= 4 * p + 3), reverse=True)
        if act:
            out.append((p, act))
    return out


def _strips_desc(p, m):
    """K-strips (q, col0, width_cols) of phase p, q descending to m."""
    out = []
    for q in range(4 * p + 3, m - 1, -1):
        c0 = max(4 * p, q) * BLK
        out.append((q, c0, (4 * p + 4) * BLK - c0))
    return out


def _b_layout(c):
    """bpack in global consumption order: phases big-first, strips q-desc,
    padded so no strip crosses a uniform chunk edge.
    Returns ({(p, q): col offset}, padded total width)."""
    edges = _b_edges()
    off, w, ei = {}, 0, 0
    for p, act in _phases(c):
        for q, _, wid in _strips_desc(p, act[-1]):
            while ei < len(edges) and w + wid > edges[ei]:
                if w < edges[ei]:
                    w = edges[ei]  # pad to the edge
                ei += 1
            off[(p, q)] = w
            w += wid
    return off, w


def _a_layout(c):
    """Packed-A slots in consumption order: everything is first used in
    the first (biggest) phase, in wavefront order: q descending, rows
    descending within a wavefront."""
    phs = _phases(c)
    kmax = 4 * phs[0][0] + 3
    rows = sorted(set(i for _, act in phs for i in act), reverse=True)
    slots = {}
    for q in range(kmax, -1, -1):
        for i in rows:
            if i <= q:
                slots[(q, i)] = len(slots)
    return slots


def _c_layout(c):
    """Packed-C 512-col slots: {(p, i): slot}, contiguous per phase."""
    slots = {}
    for p, act in _phases(c):
        for i in act:
            slots[(p, i)] = len(slots)
    return slots


NA_MAX = max(len(_a_layout(c)) for c in range(N_CORES))
WB_UNI = max(_b_layout(c)[1] for c in range(N_CORES))
NC_MAX = max(len(_c_layout(c)) for c in range(N_CORES))
B_CUTS = [0] + [e for e in _b_edges() if e < WB_UNI] + [WB_UNI]
A_CUTS = [0] + _a_cuts()


class _Chunks:
    """Resolve a packed offset to (tile, local offset)."""

    def __init__(self, cuts, tiles):
        self.cuts = cuts
        self.tiles = tiles  # tiles[k] covers [cuts[k], cuts[k+1])

    def at(self, off):
        for k in range(len(self.tiles)):
            if self.cuts[k] <= off < self.cuts[k + 1]:
                return self.tiles[k], off - self.cuts[k]
        raise AssertionError(f"offset {off} outside chunks {self.cuts}")


def _emit_preamble(nc, tc, pools, dram_io):
    """Identical for all cores, before the Switch: PE warmup + the WHOLE
    load stream.  B chunks on the SP ring, A chunks on the ACT ring: the
    rings arm in parallel and no load issue ever sits behind a Switch
    arm-entry fetch stall."""
    import concourse.mybir as mybir

    f32 = mybir.dt.float32
    bf16 = mybir.dt.bfloat16
    apool, bpool, cpool, psum_pool = pools
    apack, bpack = dram_io["apack"], dram_io["bpack"]

    warm = bpool.tile([BLK, PW], bf16, name="warm", tag="warm")
    nc.gpsimd.memset(warm[:], 0.0)
    wps = psum_pool.tile([BLK, PW], f32, name="warmps", tag="ps7")
    for i in range(N_WARM):
        nc.tensor.matmul(
            wps[:], warm[:, :BLK], warm[:], start=(i == 0), stop=(i == N_WARM - 1)
        )
    b_tiles, a_tiles = [], []
    nb, na = len(B_CUTS) - 1, len(A_CUTS) - 1
    for k in range(max(nb, na)):
        if k < nb:
            lo, hi = B_CUTS[k], B_CUTS[k + 1]
            t = bpool.tile([BLK, hi - lo], bf16, name=f"b{k}", tag=f"b{k}")
            nc.sync.dma_start(t[:], bpack[:, lo:hi])
            b_tiles.append(t)
        if k < na:
            lo, hi = A_CUTS[k], A_CUTS[k + 1]
            t = apool.tile([BLK, hi - lo, BLK], bf16, name=f"a{k}", tag=f"a{k}")
            nc.scalar.dma_start(t[:], apack[:, lo:hi, :])
            a_tiles.append(t)
    return _Chunks(A_CUTS, a_tiles), _Chunks(B_CUTS, b_tiles)


def _emit_core(nc, tc, pools, dram_io, core, ach, bch):
    import concourse.mybir as mybir

    f32 = mybir.dt.float32
    fp16 = mybir.dt.float16
    apool, bpool, cpool, psum_pool = pools
    cpack = dram_io["cpack"]
    aslot = _a_layout(core)
    cslot = _c_layout(core)
    boff, _ = _b_layout(core)
    phs = _phases(core)

    allrows = sorted(set(i for _, act in phs for i in act), reverse=True)
    bank_of = {i: k for k, i in enumerate(allrows)}  # stable across phases

    # compute: phases big-first; within a phase, strips in wavefront
    # (pack/arrival) order, each applied to all active rows.  A row's
    # region grows leftward over the diagonal strips, so each of those
    # emits an extra start=True matmul for the new 128-col sliver.  ALL
    # C stores ride SWDGE (GpSimd): its arm branches cost ~70ns, unlike
    # the Activation ring whose exit branch cost 4.7-6us on the tail.
    for pi, (p, act) in enumerate(phs):
        last_phase = pi == len(phs) - 1
        if not last_phase:
            cst = cpool.tile(
                [BLK, len(act) * PW], fp16, name=f"cst_{p}", tag=f"cst{pi}"
            )
        pst = {
            i: psum_pool.tile(
                [BLK, PW], f32, name=f"ps_{p}_{i}", tag=f"ps{bank_of[i]}"
            )
            for i in act
        }
        first = 4 * p + 3
        for q, c0, wid in _strips_desc(p, act[-1]):
            rel = c0 - 4 * p * BLK
            bt, blo = bch.at(boff[(p, q)])
            rows_in = [i for i in act if i <= q]
            rows_in = rows_in[1:] + rows_in[:1]  # retiring row (i == q) last
            for i in rows_in:
                at_, alo = ach.at(aslot[(q, i)])
                # start=True clears the whole bank's has_written bits, so
                # it is only legal on the group-opening strip; afterwards
                # the per-element bits overwrite-vs-accumulate correctly
                # as the region grows leftward over the diagonal strips.
                nc.tensor.matmul(
                    pst[i][:, rel : rel + wid],
                    at_[:, alo, :],
                    bt[:, blo : blo + wid],
                    start=(q == first),
                    stop=(q == i),
                )
            if q in act:  # row q retires: evict + store
                i = q
                mr = max(0, i - 4 * p) * BLK
                if last_phase:
                    ji = act.index(i)
                    ct = cpool.tile(
                        [BLK, PW], fp16, name=f"ct_{i}", tag=f"ct{ji % 8}"
                    )
                    nc.vector.tensor_copy(ct[:, mr:PW], pst[i][:, mr:PW])
                    nc.gpsimd.dma_start(
                        cpack[
                            :, cslot[(p, i)] * PW + mr : (cslot[(p, i)] + 1) * PW
                        ],
                        ct[:, mr:PW],
                    )
                else:
                    s0 = cslot[(p, act[0])]
                    jrow = cslot[(p, i)] - s0
                    nc.vector.tensor_copy(
                        cst[:, jrow * PW + mr : (jrow + 1) * PW], pst[i][:, mr:PW]
                    )
                    if i == act[-1]:
                        nc.gpsimd.dma_start(
                            cpack[:, s0 * PW : (s0 + len(act)) * PW], cst[:]
                        )


def _build():
    import concourse.mybir as mybir
    import concourse.tile as tile
    from concourse import bacc

    nc = bacc.Bacc(None, target_bir_lowering=False, debug=False)
    bf16 = mybir.dt.bfloat16
    fp16 = mybir.dt.float16
    with tile.TileContext(nc) as tc:
        with (
            tc.tile_pool(name="dram", bufs=1, space="DRAM") as dram,
            tc.tile_pool(name="apool", bufs=1) as apool,
            tc.tile_pool(name="bpool", bufs=1) as bpool,
            tc.tile_pool(name="cpool", bufs=1) as cpool,
            tc.tile_pool(name="psum", bufs=1, space="PSUM") as psum_pool,
        ):
            dram_io = {
                "apack": dram.tile(
                    [BLK, NA_MAX, BLK], bf16, kind="ExternalInput",
                    name="apack", uniquify=False,
                ),
                "bpack": dram.tile(
                    [BLK, WB_UNI], bf16, kind="ExternalInput",
                    name="bpack", uniquify=False,
                ),
                "cpack": dram.tile(
                    [BLK, NC_MAX * PW], fp16, kind="ExternalOutput",
                    name="cpack", uniquify=False,
                ),
            }
            pid = nc.partition_id()
            pools = (apool, bpool, cpool, psum_pool)
            arm_engines = [
                e for e in mybir.ALL_ENGINES if e.name in ("PE", "DVE", "Pool")
            ]
            tc.switch_hint({e: pid for e in arm_engines}, N_CORES, label="core")
            ach, bch = _emit_preamble(nc, tc, pools, dram_io)
            for c in tc.Switch(pid, N_CORES, hint="core"):
                _emit_core(nc, tc, pools, dram_io, c, ach, bch)
    nc.compile()
    return nc


_cached_nc = None

# Optional profiling knobs (used by test.py; harness leaves them off).
TRACE = False
TRACE_KW = {}
LAST_RESULTS = None


def _get_nc():
    global _cached_nc
    if _cached_nc is None:
        _cached_nc = _build()
    return _cached_nc


def _host_pack(A, B):
    import ml_dtypes

    bf16 = ml_dtypes.bfloat16
    AT = np.ascontiguousarray(A.T).astype(bf16)
    Bb = B.astype(bf16)
    apacks, bpacks = [], []
    for c in range(N_CORES):
        ap = np.zeros((BLK, NA_MAX, BLK), dtype=bf16)
        for (q, i), idx in _a_layout(c).items():
            ap[:, idx, :] = AT[q * BLK : (q + 1) * BLK, i * BLK : (i + 1) * BLK]
        bp = np.zeros((BLK, WB_UNI), dtype=bf16)
        boff, _ = _b_layout(c)
        for p, act in _phases(c):
            for q, c0, wid in _strips_desc(p, act[-1]):
                w0 = boff[(p, q)]
                bp[:, w0 : w0 + wid] = Bb[q * BLK : (q + 1) * BLK, c0 : c0 + wid]
        apacks.append(ap)
        bpacks.append(bp)
    return apacks, bpacks


def kernel(A, B):
    from concourse.bass_utils import run_bass_kernel_spmd

    A = np.asarray(A, dtype=np.float32)
    B = np.asarray(B, dtype=np.float32)
    nc = _get_nc()
    apacks, bpacks = _host_pack(A, B)
    in_maps = [{"apack": apacks[c], "bpack": bpacks[c]} for c in range(N_CORES)]
    res = run_bass_kernel_spmd(
        nc, in_maps, core_ids=list(range(N_CORES)), trace=TRACE, **TRACE_KW
    )
    global LAST_RESULTS
    LAST_RESULTS = res

    C = np.zeros((N, N), dtype=np.float32)
    for c in range(N_CORES):
        cp = res.results[c]["cpack"]
        for (p, i), j in _c_layout(c).items():
            mr = max(0, i - 4 * p) * BLK
            C[i * BLK : (i + 1) * BLK, p * PW + mr : (p + 1) * PW] = cp[
                :, j * PW + mr : (j + 1) * PW
            ].astype(np.float32)
    return np.triu(C)
